# revision 1
# baseline (speedup 1.0000x reference)
"""Paged-attention block (QKV proj + QK-RMSNorm + partial RoPE + paged KV attention
+ o_proj) on 8 trn2 NeuronCores, tensor-parallel over heads.

Sharding: core c owns q-heads 4c..4c+3 and kv-head c (shard qkv_weight rows /
o_proj_weight columns / kv caches by head). Each core computes a partial
o_proj output; the host sums the 8 partials (the "allreduce").

All matmuls run as float32r (1 cycle/row on PE at N>=256, ~1e-4 rel err).
"""

import numpy as np

# problem constants (hardcoded per task contract)
B, SQ, HID = 4, 512, 4096
T = B * SQ
HQ, HKV, D, R = 32, 8, 128, 64
PAGE, MAX_PAGES = 64, 16
CACHED = 512
KV_LEN = CACHED + SQ          # 1024 logical kv positions per sequence
NCORES = 8
GH = HQ // NCORES             # 4 q heads per core
KB = KV_LEN // 128            # 8 kv tiles of 128
NKB = SQ // 128               # 4 new kv tiles
EPS = 1e-6
SCALE = 1.0 / float(D) ** 0.5
NEG = -1.0e30

_COMPILED = None


def _build(no_attn=False, no_oproj=False, no_qkv_mm=False, no_dma_h=False, reps=1, oproj_per_b=False):
    import concourse.tile as tile
    from concourse import mybir, bacc
    from concourse.bass import ds, ts
    from contextlib import ExitStack

    fr = mybir.dt.float32r
    f32 = mybir.dt.float32
    X = mybir.AxisListType.X

    nc = bacc.Bacc("TRN2", target_bir_lowering=False, debug=False,
                   num_devices=NCORES)

    # hidden, host-pretiled: hTb[m, p, k*128+t] = hidden[m*128+t, k*128+p]
    hT = nc.dram_tensor("hT", (T // 128, 128, HID), fr, kind="ExternalInput")
    wqkv = nc.dram_tensor("wqkv", (HID, (GH + 2) * D), fr, kind="ExternalInput")
    wo = nc.dram_tensor("wo", (GH * D, HID), fr, kind="ExternalInput")
    kcT = nc.dram_tensor("kcT", (B, D, CACHED), fr, kind="ExternalInput")
    vc = nc.dram_tensor("vc", (B, CACHED, D), fr, kind="ExternalInput")
    cosel = nc.dram_tensor("cosel", (T, R // 2), f32, kind="ExternalInput")
    sinel = nc.dram_tensor("sinel", (T, R // 2), f32, kind="ExternalInput")
    trimask = nc.dram_tensor("trimask", (128, 128), f32, kind="ExternalInput")
    mask3 = nc.dram_tensor("mask3", (128, 256), f32, kind="ExternalInput")
    ident = nc.dram_tensor("ident", (128, 128), f32, kind="ExternalInput")
    onesd = nc.dram_tensor("onesd", (128, 128), fr, kind="ExternalInput")
    outp = nc.dram_tensor("outp", (T, HID), f32, kind="ExternalOutput")

    NF = (GH + 2) * D          # 768 qkv features per core
    NQ = GH * D                # 512 (q features)
    NH = GH + 1                # 5 normed+roped heads (4 q + 1 k)

    with tile.TileContext(nc) as tc, ExitStack() as ctx:
        persist = ctx.enter_context(tc.tile_pool(name="persist", bufs=1))
        qt_pool = ctx.enter_context(tc.tile_pool(name="qt", bufs=2))
        kt_pool = ctx.enter_context(tc.tile_pool(name="kt", bufs=2))
        at_pool = ctx.enter_context(tc.tile_pool(name="at", bufs=2 if oproj_per_b else B))
        work = ctx.enter_context(tc.tile_pool(name="work", bufs=2))
        scratch = ctx.enter_context(tc.tile_pool(name="scratch", bufs=1))
        ps = ctx.enter_context(tc.tile_pool(name="ps", bufs=8, space="PSUM"))

        ident_sb = persist.tile([128, 128], f32, tag="ident")
        nc.sync.dma_start(ident_sb[:], ident[:])
        tri_sb = persist.tile([128, 128], f32, tag="tri")
        nc.sync.dma_start(tri_sb[:], trimask[:])
        m3_sb = persist.tile([128, 256], f32, tag="m3")
        nc.sync.dma_start(m3_sb[:], mask3[:])
        ones_sb = persist.tile([128, 128], fr, tag="ones")
        nc.sync.dma_start(ones_sb[:], onesd[:])
        eps_sb = persist.tile([128, 1], f32, tag="eps")
        nc.vector.memset(eps_sb[:], EPS)

        for _rep in range(reps):
            attnT = []  # per-seq [128(d), GH, 512(q)] attention outputs (o_proj lhsT)
            wo_ap = wo[:].rearrange("(ko p) f -> p ko f", p=128)

            with ExitStack() as rctx:
                if oproj_per_b:
                    opool = rctx.enter_context(tc.tile_pool(name="oproj", bufs=2))
                    outpool = rctx.enter_context(tc.tile_pool(name="outstage", bufs=2))
                qph = rctx.enter_context(tc.tile_pool(name="qkvph", bufs=1))
                hpool = rctx.enter_context(tc.tile_pool(name="hstream", bufs=3))
                # resident qkv weights [128, 32(k), 768]
                wq_sb = qph.tile([128, HID // 128, NF], fr, tag="wq")
                wq_ap = wqkv[:].rearrange("(ko p) f -> p ko f", p=128)
                for kq in range(8):
                    nc.sync.dma_start(wq_sb[:, ts(kq, 4), :], wq_ap[:, ts(kq, 4), :])

                hT_ap = hT[:].rearrange("m p (ko t) -> m p ko t", t=128)

                for b in range(B):
                    QT_b = qt_pool.tile([128, GH, SQ], fr, tag="QT")
                    KT_b = kt_pool.tile([128, SQ], fr, tag="KT")
                    V_b = kt_pool.tile([128, NKB, 128], fr, tag="Vnew")
                    kcT_b = kt_pool.tile([128, CACHED], fr, tag="kcT")
                    nc.sync.dma_start(kcT_b[:], kcT[b].rearrange("p k -> p k"))
                    vc_b = kt_pool.tile([128, NKB, 128], fr, tag="vc")
                    nc.sync.dma_start(vc_b[:], vc[b].rearrange("(blk p) d -> p blk d", p=128))

                    # Pipelined per token tile: matmuls for tile ml run
                    # first; the PE transposes for tile ml-1 are emitted after
                    # them, so PE never waits on the DVE/ACT norm+rope chain
                    # of the tile it just produced.
                    half = R // 2  # 32
                    mult = mybir.AluOpType.mult

                    def emit_transposes(ml, qkv_sb):
                        for h5 in range(NH):
                            pst = ps.tile([128, 512], f32, tag="ps", name="pst")
                            nc.tensor.transpose(pst[:, 0:128], qkv_sb[:, ts(h5, D)],
                                                ident_sb[:])
                            if h5 < GH:
                                nc.any.tensor_copy(QT_b[:, h5, ds(ml * 128, 128)],
                                                   pst[:, 0:128])
                            else:
                                nc.any.tensor_copy(KT_b[:, ds(ml * 128, 128)],
                                                   pst[:, 0:128])

                    prev = None
                    for ml in range(NKB):
                        m = b * NKB + ml
                        ht_t = hpool.tile([128, 16, 128], fr, tag="ht")
                        ht_t2 = hpool.tile([128, 16, 128], fr, tag="ht")
                        if not no_dma_h:
                            nc.sync.dma_start(ht_t[:], hT_ap[m, :, 0:16, :])
                            nc.sync.dma_start(ht_t2[:], hT_ap[m, :, 16:32, :])
                        cos_sb = work.tile([128, R // 2], f32, tag="cos", bufs=2)
                        sin_sb = work.tile([128, R // 2], f32, tag="sin", bufs=2)
                        nc.sync.dma_start(cos_sb[:], cosel[ds(m * 128, 128), :])
                        nc.sync.dma_start(sin_sb[:], sinel[ds(m * 128, 128), :])

                        # qkv projection: out [tokens(128), features(768)]
                        ps_hi = ps.tile([128, 512], f32, tag="ps")
                        ps_lo = ps.tile([128, 512], f32, tag="ps")
                        nk = HID // 128
                        if no_qkv_mm:
                            nk = 1
                        for k in range(nk):
                            src = ht_t[:, k, :] if k < 16 else ht_t2[:, k - 16, :]
                            nc.tensor.matmul(ps_hi[:], src, wq_sb[:, k, 0:512],
                                             start=(k == 0), stop=(k == nk - 1))
                            nc.tensor.matmul(ps_lo[:, 0:NF - 512], src,
                                             wq_sb[:, k, 512:NF],
                                             start=(k == 0), stop=(k == nk - 1))

                        if prev is not None:
                            emit_transposes(prev[0], prev[1])

                        # RMSNorm stats straight from PSUM
                        x2 = scratch.tile([128, NH * D], f32, tag="x2")
                        nc.scalar.square(x2[:, 0:512], ps_hi[:])
                        nc.scalar.square(x2[:, 512:NH * D], ps_lo[:, 0:128])
                        ss = work.tile([128, NH], f32, tag="ss")
                        nc.vector.reduce_sum(out=ss[:], in_=x2[:].rearrange(
                            "p (h d) -> p h d", h=NH), axis=X)
                        nc.scalar.activation(ss[:], ss[:],
                                             mybir.ActivationFunctionType.Sqrt,
                                             bias=eps_sb[:], scale=1.0 / D)
                        rstd = work.tile([128, NH], f32, tag="rstd")
                        nc.vector.reciprocal(rstd[:], ss[:])
                        # normalize PSUM -> qkv_sb (q heads + k); copy v out
                        qkv_sb = work.tile([128, NH * D], f32, tag="qkv_sb", bufs=3)
                        for h5 in range(NH):
                            src_ap = ps_hi[:, ts(h5, D)] if h5 < GH else \
                                ps_lo[:, 0:128]
                            nc.vector.tensor_scalar_mul(
                                qkv_sb[:, ts(h5, D)], src_ap, rstd[:, ds(h5, 1)])
                        nc.any.tensor_copy(V_b[:, ml, :], ps_lo[:, 128:256])

                        # rope (DVE) in place on qkv_sb
                        v3 = qkv_sb[:].rearrange("p (h d) -> p h d", h=NH)
                        x1v = v3[:, :, 0:half]
                        x2v = v3[:, :, half:R]
                        cb = cos_sb[:, None, :].to_broadcast((128, NH, half))
                        sb_ = sin_sb[:, None, :].to_broadcast((128, NH, half))
                        t1 = scratch.tile([128, NH, half], f32, tag="t1")
                        t2 = scratch.tile([128, NH, half], f32, tag="t2")
                        t3 = scratch.tile([128, NH, half], f32, tag="t3")
                        t4 = scratch.tile([128, NH, half], f32, tag="t4")
                        nc.vector.tensor_tensor(t1[:], x1v, cb, mult)
                        nc.vector.tensor_tensor(t2[:], x2v, sb_, mult)
                        nc.vector.tensor_tensor(t3[:], x1v, sb_, mult)
                        nc.vector.tensor_tensor(t4[:], x2v, cb, mult)
                        nc.vector.tensor_tensor(x1v, t1[:], t2[:],
                                                mybir.AluOpType.subtract)
                        nc.vector.tensor_tensor(x2v, t3[:], t4[:],
                                                mybir.AluOpType.add)
                        prev = (ml, qkv_sb)
                    emit_transposes(prev[0], prev[1])

                    # ---- attention for sequence b ----
                    aT = at_pool.tile([128, GH, SQ], fr, tag="attnT")
                    attnT.append(aT)
                    for h in range(0 if not no_attn else GH, GH):
                        outT_ps = ps.tile([128, 512], f32, tag="ps")
                        den_ps = ps.tile([128, 512], f32, tag="ps")
                        for t in range(KB):
                            off = 0 if t < 4 else min((t - 4) * 128, 256)
                            N = SQ - off
                            lhsT = kcT_b[:, ts(t, 128)] if t < 4 else \
                                KT_b[:, ts(t - 4, 128)]
                            vt = vc_b[:, t, :] if t < 4 else V_b[:, t - 4, :]
                            sc_ps = ps.tile([128, 512], f32, tag="ps")
                            nc.tensor.matmul(sc_ps[:, off:SQ], lhsT, QT_b[:, h, off:SQ],
                                             start=True, stop=True)
                            if t == KB - 1:
                                nc.vector.tensor_tensor(sc_ps[:, 256:512],
                                                        sc_ps[:, 256:512],
                                                        m3_sb[:], mybir.AluOpType.add)
                            elif t >= 4:
                                nc.vector.tensor_tensor(sc_ps[:, ds((t - 4) * 128, 128)],
                                                        sc_ps[:, ds((t - 4) * 128, 128)],
                                                        tri_sb[:], mybir.AluOpType.add)
                            e_t = work.tile([128, 512], fr, tag="e")
                            nc.scalar.activation(e_t[:, 0:N], sc_ps[:, off:SQ],
                                                 mybir.ActivationFunctionType.Exp,
                                                 scale=SCALE)
                            nc.tensor.matmul(outT_ps[:, off:SQ], vt, e_t[:, 0:N],
                                             start=(t == 0), stop=(t == KB - 1),
                                             skip_group_check=True)
                            nc.tensor.matmul(den_ps[:, off:SQ], ones_sb[:], e_t[:, 0:N],
                                             start=(t == 0), stop=(t == KB - 1),
                                             skip_group_check=True)
                        recip = scratch.tile([128, 512], f32, tag="recip")
                        nc.vector.reciprocal(recip[:], den_ps[:])
                        nc.vector.tensor_tensor(aT[:, h, :], outT_ps[:], recip[:],
                                                mybir.AluOpType.mult)

                    if oproj_per_b and not no_oproj:
                        for n in range(HID // 512):
                            wo_na = opool.tile([128, 2, 512], fr, tag="wo_n")
                            wo_nb = opool.tile([128, 2, 512], fr, tag="wo_n")
                            nc.sync.dma_start(wo_na[:], wo_ap[:, 0:2, ds(n * 512, 512)])
                            nc.sync.dma_start(wo_nb[:], wo_ap[:, 2:4, ds(n * 512, 512)])
                            for ml in range(NKB):
                                po = ps.tile([128, 512], f32, tag="ps")
                                for h in range(GH):
                                    wsrc = wo_na[:, h, :] if h < 2 else wo_nb[:, h - 2, :]
                                    nc.tensor.matmul(po[:], aT[:, h, ts(ml, 128)],
                                                     wsrc,
                                                     start=(h == 0), stop=(h == GH - 1))
                                ob = outpool.tile([128, 512], f32, tag="ob")
                                if (ml + n) % 2 == 0:
                                    nc.vector.tensor_copy(ob[:], po[:])
                                else:
                                    nc.scalar.copy(ob[:], po[:])
                                nc.sync.dma_start(
                                    outp[ds((b * NKB + ml) * 128, 128),
                                         ds(n * 512, 512)], ob[:])

            # ---- o_proj (phase-3 variant): partial = attnT.T @ woT ----
            if oproj_per_b:
                continue
            with tc.tile_pool(name="oproj", bufs=2) as opool, \
                 tc.tile_pool(name="outstage", bufs=3) as outpool:
                for n in range(0 if not no_oproj else HID // 512, HID // 512):
                    wo_n = opool.tile([128, GH, 512], fr, tag="wo_n")
                    nc.sync.dma_start(wo_n[:], wo_ap[:, :, ds(n * 512, 512)])
                    for b in range(B):
                        for ml in range(NKB):
                            po = ps.tile([128, 512], f32, tag="ps")
                            for h in range(GH):
                                nc.tensor.matmul(po[:], attnT[b][:, h, ts(ml, 128)],
                                                 wo_n[:, h, :],
                                                 start=(h == 0), stop=(h == GH - 1))
                            ob = outpool.tile([128, 512], f32, tag="ob")
                            if (b * NKB + ml) % 2 == 0:
                                nc.vector.tensor_copy(ob[:], po[:])
                            else:
                                nc.scalar.copy(ob[:], po[:])
                            nc.sync.dma_start(
                                outp[ds((b * NKB + ml) * 128, 128), ds(n * 512, 512)],
                                ob[:])

    nc.compile()
    return nc


def _get_compiled():
    global _COMPILED
    if _COMPILED is None:
        _COMPILED = _build()
    return _COMPILED


def _prep_inputs(hidden_states, cos, sin, positions, k_cache, v_cache, page_table,
                 cache_seqlens, cu_seqlens_q, qkv_weight, o_proj_weight,
                 q_norm_weight, k_norm_weight):
    f32 = np.float32
    pos = np.asarray(positions).reshape(B, SQ)
    assert np.array_equal(np.asarray(cache_seqlens),
                          np.full(B, CACHED, np.int32)), "cache_seqlens != CACHED"
    assert np.array_equal(np.asarray(cu_seqlens_q),
                          np.arange(B + 1, dtype=np.int64) * SQ), "cu_seqlens ragged"
    assert (pos == CACHED + np.arange(SQ)[None, :]).all(), "positions ragged"
    assert np.allclose(q_norm_weight, 1.0) and np.allclose(k_norm_weight, 1.0), \
        "non-unit norm weights unsupported"

    pt = np.asarray(page_table)
    phys = (pt[:, :, None] * PAGE + np.arange(PAGE)[None, None, :]).reshape(B, -1)
    slots = pt[np.arange(B)[:, None], pos // PAGE] * PAGE + pos % PAGE
    assert np.array_equal(slots, phys[:, CACHED:]), "non-append page layout"

    kf = np.asarray(k_cache).reshape(-1, HKV, D)
    vf = np.asarray(v_cache).reshape(-1, HKV, D)
    Kc = kf[phys[:, :CACHED]]          # [B, 512, HKV, D]
    Vc = vf[phys[:, :CACHED]]

    cos_sel = np.ascontiguousarray(np.asarray(cos)[positions], dtype=f32)
    sin_sel = np.ascontiguousarray(np.asarray(sin)[positions], dtype=f32)
    # hTb[m, p, k*128+t] = hidden[m*128+t, k*128+p]
    hT = np.ascontiguousarray(
        np.asarray(hidden_states, dtype=f32).reshape(T // 128, 128, HID // 128, 128)
        .transpose(0, 3, 2, 1).reshape(T // 128, 128, HID))
    tri = np.where(np.arange(128)[None, :] >= np.arange(128)[:, None],
                   np.float32(0.0), np.float32(NEG))
    m3 = np.concatenate([np.full((128, 128), NEG, f32), tri], axis=1)
    eye = np.eye(128, dtype=f32)

    qw = np.asarray(qkv_weight)
    ow = np.asarray(o_proj_weight)
    in_maps = []
    for c in range(NCORES):
        rows = np.concatenate([
            qw[c * GH * D:(c + 1) * GH * D],
            qw[HQ * D + c * D: HQ * D + (c + 1) * D],
            qw[HQ * D + HKV * D + c * D: HQ * D + HKV * D + (c + 1) * D],
        ], axis=0)
        in_maps.append(dict(
            hT=hT,
            wqkv=np.ascontiguousarray(rows.T, dtype=f32),
            wo=np.ascontiguousarray(ow[:, c * GH * D:(c + 1) * GH * D].T, dtype=f32),
            kcT=np.ascontiguousarray(Kc[:, :, c, :].transpose(0, 2, 1), dtype=f32),
            vc=np.ascontiguousarray(Vc[:, :, c, :], dtype=f32),
            cosel=cos_sel, sinel=sin_sel, trimask=tri, ident=eye, mask3=m3,
            onesd=np.ones((128, 128), dtype=f32),
        ))
    return in_maps


def kernel(**inputs) -> np.ndarray:
    from concourse.bass_utils import run_bass_kernel_spmd
    in_maps = _prep_inputs(**inputs)
    nc = _get_compiled()
    res = run_bass_kernel_spmd(nc, in_maps, core_ids=list(range(NCORES)))
    acc = res.results[0]["outp"].astype(np.float32).copy()
    for c in range(1, NCORES):
        acc += res.results[c]["outp"]
    return acc



# revision 7
# speedup vs baseline: 185.9326x; 185.9326x over previous
"""Paged-attention block (QKV proj + QK-RMSNorm + partial RoPE + paged KV attention
+ o_proj) on 8 trn2 NeuronCores, tensor-parallel over heads.

Sharding: core c owns q-heads 4c..4c+3 and kv-head c (shard qkv_weight rows /
o_proj_weight columns / kv caches by head). Each core computes a partial
o_proj output; the host sums the 8 partials (the "allreduce").

All matmul operands are bf16 (same 1 row/cycle PE rate as fp32r, half the
DMA/SBUF footprint, fast weight loads); PSUM accumulation stays fp32.
The attention inner loop is software-pipelined: the scores matmul runs 3
steps ahead of the PV/denominator matmuls so the PE never waits on the
ACT-engine exp.
"""

import numpy as np

# problem constants (hardcoded per task contract)
B, SQ, HID = 4, 512, 4096
T = B * SQ
HQ, HKV, D, R = 32, 8, 128, 64
PAGE, MAX_PAGES = 64, 16
CACHED = 512
KV_LEN = CACHED + SQ          # 1024 logical kv positions per sequence
NCORES = 8
GH = HQ // NCORES             # 4 q heads per core
KB = KV_LEN // 128            # 8 kv tiles of 128
NKB = SQ // 128               # 4 new kv tiles
EPS = 1e-6
SCALE = 1.0 / float(D) ** 0.5
NEG = -1.0e30

_COMPILED = None


def _build(reps=1):
    import concourse.tile as tile
    from concourse import mybir, bacc
    from concourse.bass import ds, ts
    from contextlib import ExitStack

    bf = mybir.dt.bfloat16
    f32 = mybir.dt.float32

    nc = bacc.Bacc("TRN2", target_bir_lowering=False, debug=False,
                   num_devices=NCORES)

    # hidden, host-pretiled: hTb[m, p, k*128+t] = hidden[m*128+t, k*128+p]
    hT = nc.dram_tensor("hT", (T // 128, 128, HID), bf, kind="ExternalInput")
    wqkv = nc.dram_tensor("wqkv", (HID, (GH + 2) * D), bf, kind="ExternalInput")
    wo = nc.dram_tensor("wo", (GH * D, HID), bf, kind="ExternalInput")
    kcT = nc.dram_tensor("kcT", (B, D, CACHED), bf, kind="ExternalInput")
    vc = nc.dram_tensor("vc", (B, CACHED, D), bf, kind="ExternalInput")
    cosel = nc.dram_tensor("cosel", (T, R // 2), bf, kind="ExternalInput")
    sinel = nc.dram_tensor("sinel", (T, R // 2), bf, kind="ExternalInput")
    trimask = nc.dram_tensor("trimask", (128, 128), f32, kind="ExternalInput")
    mask3 = nc.dram_tensor("mask3", (128, 256), f32, kind="ExternalInput")
    ident = nc.dram_tensor("ident", (128, 128), bf, kind="ExternalInput")
    onesd = nc.dram_tensor("onesd", (128, 128), bf, kind="ExternalInput")
    outp = nc.dram_tensor("outp", (T, HID), f32, kind="ExternalOutput")

    NF = (GH + 2) * D          # 768 qkv features per core
    NH = GH + 1                # 5 normed+roped heads (4 q + 1 k)

    with tile.TileContext(nc) as tc, ExitStack() as ctx:
        persist = ctx.enter_context(tc.tile_pool(name="persist", bufs=1))
        qt_pool = ctx.enter_context(tc.tile_pool(name="qt", bufs=2))
        kt_pool = ctx.enter_context(tc.tile_pool(name="kt", bufs=2))
        at_pool = ctx.enter_context(tc.tile_pool(name="at", bufs=B))
        work = ctx.enter_context(tc.tile_pool(name="work", bufs=2))
        scratch = ctx.enter_context(tc.tile_pool(name="scratch", bufs=1))
        ps = ctx.enter_context(tc.tile_pool(name="ps", bufs=8, space="PSUM"))

        ident_sb = persist.tile([128, 128], bf, tag="ident")
        nc.sync.dma_start(ident_sb[:], ident[:])
        tri_sb = persist.tile([128, 128], f32, tag="tri")
        nc.sync.dma_start(tri_sb[:], trimask[:])
        m3_sb = persist.tile([128, 256], f32, tag="m3")
        nc.sync.dma_start(m3_sb[:], mask3[:])
        ones_sb = persist.tile([128, 128], bf, tag="ones")
        nc.sync.dma_start(ones_sb[:], onesd[:])
        eps_sb = persist.tile([128, 1], f32, tag="eps")
        nc.vector.memset(eps_sb[:], EPS)

        half = R // 2  # 32
        mult = mybir.AluOpType.mult

        for _rep in range(reps):
            attnT = []  # per-seq [128(d), GH, 512(q)] attention outputs (o_proj lhsT)
            wo_ap = wo[:].rearrange("(ko p) f -> p ko f", p=128)

            with ExitStack() as rctx:
                qph = rctx.enter_context(tc.tile_pool(name="qkvph", bufs=1))
                hpool = rctx.enter_context(tc.tile_pool(name="hstream", bufs=3))
                # resident qkv weights [128, 32(k), 768]
                wq_sb = qph.tile([128, HID // 128, NF], bf, tag="wq")
                wq_ap = wqkv[:].rearrange("(ko p) f -> p ko f", p=128)
                for kq in range(8):
                    nc.sync.dma_start(wq_sb[:, ts(kq, 4), :], wq_ap[:, ts(kq, 4), :])

                hT_ap = hT[:].rearrange("m p (ko t) -> m p ko t", t=128)

                for b in range(B):
                    QT_b = qt_pool.tile([128, GH, SQ], bf, tag="QT")
                    KT_b = kt_pool.tile([128, SQ], bf, tag="KT")
                    V_b = kt_pool.tile([128, NKB, 128], bf, tag="Vnew")
                    kcT_b = kt_pool.tile([128, CACHED], bf, tag="kcT")
                    nc.sync.dma_start(kcT_b[:], kcT[b].rearrange("p k -> p k"))
                    vc_b = kt_pool.tile([128, NKB, 128], bf, tag="vc")
                    nc.sync.dma_start(vc_b[:], vc[b].rearrange("(blk p) d -> p blk d", p=128))

                    # PE transposes for token tile ml: 4 q heads batched into one
                    # PSUM bank, k into a second; single strided copies out.
                    def emit_transposes(ml, qkv_sb):
                        pq = ps.tile([128, 512], bf, tag="ps", name="pq")
                        pk = ps.tile([128, 128], bf, tag="ps", name="pk")
                        for h in range(GH):
                            nc.tensor.transpose(pq[:, ts(h, 128)],
                                                qkv_sb[:, ts(h, D)], ident_sb[:])
                        nc.tensor.transpose(pk[:], qkv_sb[:, ts(GH, D)],
                                            ident_sb[:])
                        nc.vector.tensor_copy(
                            QT_b[:, :, ds(ml * 128, 128)],
                            pq[:].rearrange("p (h t) -> p h t", h=GH))
                        nc.scalar.copy(KT_b[:, ds(ml * 128, 128)], pk[:])

                    # Pipelined per token tile: matmuls for tile ml run first;
                    # the PE transposes for tile ml-1 are emitted after them,
                    # so PE never waits on the DVE/ACT norm+rope chain.
                    prev = None
                    for ml in range(NKB):
                        m = b * NKB + ml
                        ht_t = hpool.tile([128, 16, 128], bf, tag="ht")
                        ht_t2 = hpool.tile([128, 16, 128], bf, tag="ht")
                        nc.sync.dma_start(ht_t[:], hT_ap[m, :, 0:16, :])
                        nc.sync.dma_start(ht_t2[:], hT_ap[m, :, 16:32, :])
                        cos_sb = work.tile([128, half], bf, tag="cos", bufs=2)
                        sin_sb = work.tile([128, half], bf, tag="sin", bufs=2)
                        nc.sync.dma_start(cos_sb[:], cosel[ds(m * 128, 128), :])
                        nc.sync.dma_start(sin_sb[:], sinel[ds(m * 128, 128), :])

                        # qkv projection: out [tokens(128), features(768)]
                        ps_hi = ps.tile([128, 512], f32, tag="ps")
                        ps_lo = ps.tile([128, 512], f32, tag="ps")
                        for k in range(HID // 128):
                            src = ht_t[:, k, :] if k < 16 else ht_t2[:, k - 16, :]
                            nc.tensor.matmul(ps_hi[:], src, wq_sb[:, k, 0:512],
                                             start=(k == 0), stop=(k == 31))
                            nc.tensor.matmul(ps_lo[:, 0:NF - 512], src,
                                             wq_sb[:, k, 512:NF],
                                             start=(k == 0), stop=(k == 31))

                        if prev is not None:
                            emit_transposes(prev[0], prev[1])

                        # RMSNorm stats straight from PSUM (ACT square w/ accum)
                        x2 = scratch.tile([128, D], f32, tag="x2")
                        ss = work.tile([128, NH], f32, tag="ss")
                        for h5 in range(NH):
                            src_ap = ps_hi[:, ts(h5, D)] if h5 < GH else \
                                ps_lo[:, 0:128]
                            nc.scalar.activation(
                                x2[:], src_ap, mybir.ActivationFunctionType.Square,
                                accum_out=ss[:, ds(h5, 1)])
                        nc.scalar.activation(ss[:], ss[:],
                                             mybir.ActivationFunctionType.Sqrt,
                                             bias=eps_sb[:], scale=1.0 / D)
                        rstd = work.tile([128, NH], f32, tag="rstd")
                        nc.vector.reciprocal(rstd[:], ss[:])
                        # normalize PSUM -> qkv_sb (q heads + k); copy v out
                        qkv_sb = work.tile([128, NH * D], bf, tag="qkv_sb", bufs=3)
                        for h5 in range(NH):
                            src_ap = ps_hi[:, ts(h5, D)] if h5 < GH else \
                                ps_lo[:, 0:128]
                            nc.vector.tensor_scalar_mul(
                                qkv_sb[:, ts(h5, D)], src_ap, rstd[:, ds(h5, 1)])
                        nc.scalar.copy(V_b[:, ml, :], ps_lo[:, 128:256])

                        # rope (DVE) in place on qkv_sb
                        v3 = qkv_sb[:].rearrange("p (h d) -> p h d", h=NH)
                        x1v = v3[:, :, 0:half]
                        x2v = v3[:, :, half:R]
                        cb = cos_sb[:, None, :].to_broadcast((128, NH, half))
                        sb_ = sin_sb[:, None, :].to_broadcast((128, NH, half))
                        t1 = scratch.tile([128, NH, half], f32, tag="t1")
                        t2 = scratch.tile([128, NH, half], f32, tag="t2")
                        t3 = scratch.tile([128, NH, half], f32, tag="t3")
                        t4 = scratch.tile([128, NH, half], f32, tag="t4")
                        nc.vector.tensor_tensor(t1[:], x1v, cb, mult)
                        nc.vector.tensor_tensor(t2[:], x2v, sb_, mult)
                        nc.vector.tensor_tensor(t3[:], x1v, sb_, mult)
                        nc.vector.tensor_tensor(t4[:], x2v, cb, mult)
                        nc.vector.tensor_tensor(x1v, t1[:], t2[:],
                                                mybir.AluOpType.subtract)
                        nc.vector.tensor_tensor(x2v, t3[:], t4[:],
                                                mybir.AluOpType.add)
                        prev = (ml, qkv_sb)
                    emit_transposes(prev[0], prev[1])

                    # ---- attention for sequence b (software-pipelined) ----
                    aT = at_pool.tile([128, GH, SQ], bf, tag="attnT")
                    attnT.append(aT)

                    steps = [(h, t) for h in range(GH) for t in range(KB)]
                    NS = len(steps)            # 32
                    LOOK = 3
                    e_tiles = [None] * NS
                    offs, lens = [], []
                    for h, t in steps:
                        off = 0 if t <= 4 else min((t - 4) * 128, 256)
                        offs.append(off)
                        lens.append(SQ - off)
                    acc = {}                   # h -> (outT_ps, den_ps)

                    def stage_A(s):
                        h, t = steps[s]
                        off, N = offs[s], lens[s]
                        lhsT = kcT_b[:, ts(t, 128)] if t < 4 else \
                            KT_b[:, ts(t - 4, 128)]
                        sc_ps = ps.tile([128, 512], f32, tag="ps", name="sc")
                        nc.tensor.matmul(sc_ps[:, off:SQ], lhsT,
                                         QT_b[:, h, off:SQ],
                                         start=True, stop=True)
                        if t == KB - 1:
                            nc.vector.tensor_tensor(sc_ps[:, 256:512],
                                                    sc_ps[:, 256:512],
                                                    m3_sb[:], mybir.AluOpType.add)
                        elif t >= 4:
                            nc.vector.tensor_tensor(sc_ps[:, ds((t - 4) * 128, 128)],
                                                    sc_ps[:, ds((t - 4) * 128, 128)],
                                                    tri_sb[:], mybir.AluOpType.add)
                        e_t = work.tile([128, 512], bf, tag="e", bufs=5)
                        nc.scalar.activation(e_t[:, 0:N], sc_ps[:, off:SQ],
                                             mybir.ActivationFunctionType.Exp,
                                             scale=SCALE)
                        e_tiles[s] = e_t

                    def stage_C(s):
                        h, t = steps[s]
                        off, N = offs[s], lens[s]
                        if t == 0:
                            outT_ps = ps.tile([128, 512], f32, tag="ps", name="o")
                            den_ps = ps.tile([128, 512], f32, tag="ps", name="d")
                            acc[h] = (outT_ps, den_ps)
                        outT_ps, den_ps = acc[h]
                        vt = vc_b[:, t, :] if t < 4 else V_b[:, t - 4, :]
                        e_t = e_tiles[s]
                        e_tiles[s] = None
                        nc.tensor.matmul(outT_ps[:, off:SQ], vt, e_t[:, 0:N],
                                         start=(t == 0), stop=(t == KB - 1),
                                         skip_group_check=True)
                        nc.tensor.matmul(den_ps[:, off:SQ], ones_sb[:], e_t[:, 0:N],
                                         start=(t == 0), stop=(t == KB - 1),
                                         skip_group_check=True)
                        if t == KB - 1:
                            recip = scratch.tile([128, 512], f32, tag="recip",
                                                 bufs=2)
                            nc.vector.reciprocal(recip[:], den_ps[:])
                            nc.vector.tensor_tensor(aT[:, h, :], outT_ps[:],
                                                    recip[:], mult)

                    for s in range(LOOK):
                        stage_A(s)
                    for s in range(NS):
                        stage_C(s)
                        if s + LOOK < NS:
                            stage_A(s + LOOK)

            # ---- o_proj: partial = attnT.T @ woT ----
            with tc.tile_pool(name="oproj", bufs=2) as opool, \
                 tc.tile_pool(name="outstage", bufs=3) as outpool:
                for n in range(HID // 512):
                    wo_n = opool.tile([128, GH, 512], bf, tag="wo_n")
                    nc.sync.dma_start(wo_n[:], wo_ap[:, :, ds(n * 512, 512)])
                    for b in range(B):
                        for ml in range(NKB):
                            po = ps.tile([128, 512], f32, tag="ps")
                            for h in range(GH):
                                nc.tensor.matmul(po[:], attnT[b][:, h, ts(ml, 128)],
                                                 wo_n[:, h, :],
                                                 start=(h == 0), stop=(h == GH - 1))
                            ob = outpool.tile([128, 512], f32, tag="ob")
                            if (b * NKB + ml) % 2 == 0:
                                nc.vector.tensor_copy(ob[:], po[:])
                            else:
                                nc.scalar.copy(ob[:], po[:])
                            nc.sync.dma_start(
                                outp[ds((b * NKB + ml) * 128, 128), ds(n * 512, 512)],
                                ob[:])

    nc.compile()
    return nc


def _get_compiled():
    global _COMPILED
    if _COMPILED is None:
        _COMPILED = _build()
    return _COMPILED


def _prep_inputs(hidden_states, cos, sin, positions, k_cache, v_cache, page_table,
                 cache_seqlens, cu_seqlens_q, qkv_weight, o_proj_weight,
                 q_norm_weight, k_norm_weight):
    import ml_dtypes
    bf16 = ml_dtypes.bfloat16
    f32 = np.float32
    pos = np.asarray(positions).reshape(B, SQ)
    assert np.array_equal(np.asarray(cache_seqlens),
                          np.full(B, CACHED, np.int32)), "cache_seqlens != CACHED"
    assert np.array_equal(np.asarray(cu_seqlens_q),
                          np.arange(B + 1, dtype=np.int64) * SQ), "cu_seqlens ragged"
    assert (pos == CACHED + np.arange(SQ)[None, :]).all(), "positions ragged"
    assert np.allclose(q_norm_weight, 1.0) and np.allclose(k_norm_weight, 1.0), \
        "non-unit norm weights unsupported"

    pt = np.asarray(page_table)
    phys = (pt[:, :, None] * PAGE + np.arange(PAGE)[None, None, :]).reshape(B, -1)
    slots = pt[np.arange(B)[:, None], pos // PAGE] * PAGE + pos % PAGE
    assert np.array_equal(slots, phys[:, CACHED:]), "non-append page layout"

    kf = np.asarray(k_cache).reshape(-1, HKV, D)
    vf = np.asarray(v_cache).reshape(-1, HKV, D)
    Kc = kf[phys[:, :CACHED]]          # [B, 512, HKV, D]
    Vc = vf[phys[:, :CACHED]]

    cos_sel = np.ascontiguousarray(np.asarray(cos)[positions]).astype(bf16)
    sin_sel = np.ascontiguousarray(np.asarray(sin)[positions]).astype(bf16)
    # hTb[m, p, k*128+t] = hidden[m*128+t, k*128+p]
    hT = np.ascontiguousarray(
        np.asarray(hidden_states, dtype=f32).reshape(T // 128, 128, HID // 128, 128)
        .transpose(0, 3, 2, 1).reshape(T // 128, 128, HID)).astype(bf16)
    tri = np.where(np.arange(128)[None, :] >= np.arange(128)[:, None],
                   np.float32(0.0), np.float32(NEG))
    m3 = np.concatenate([np.full((128, 128), NEG, f32), tri], axis=1)
    eye = np.eye(128, dtype=bf16)

    qw = np.asarray(qkv_weight)
    ow = np.asarray(o_proj_weight)
    in_maps = []
    for c in range(NCORES):
        rows = np.concatenate([
            qw[c * GH * D:(c + 1) * GH * D],
            qw[HQ * D + c * D: HQ * D + (c + 1) * D],
            qw[HQ * D + HKV * D + c * D: HQ * D + HKV * D + (c + 1) * D],
        ], axis=0)
        in_maps.append(dict(
            hT=hT,
            wqkv=np.ascontiguousarray(rows.T).astype(bf16),
            wo=np.ascontiguousarray(ow[:, c * GH * D:(c + 1) * GH * D].T).astype(bf16),
            kcT=np.ascontiguousarray(Kc[:, :, c, :].transpose(0, 2, 1)).astype(bf16),
            vc=np.ascontiguousarray(Vc[:, :, c, :]).astype(bf16),
            cosel=cos_sel, sinel=sin_sel, trimask=tri, mask3=m3,
            ident=eye, onesd=np.ones((128, 128), dtype=bf16),
        ))
    return in_maps


def kernel(**inputs) -> np.ndarray:
    from concourse.bass_utils import run_bass_kernel_spmd
    in_maps = _prep_inputs(**inputs)
    nc = _get_compiled()
    res = run_bass_kernel_spmd(nc, in_maps, core_ids=list(range(NCORES)))
    acc = res.results[0]["outp"].astype(np.float32).copy()
    for c in range(1, NCORES):
        acc += res.results[c]["outp"]
    return acc
